# revision 5
# baseline (speedup 1.0000x reference)
"""GATv2 attention layer (B=2, T=1024, C_IN=128, D=64) on 8 trn2 NeuronCores.

Sharding: flatten (B, T) destination rows -> 2048 rows, 256 per core.
Each core gets: feat of its batch [1024,128], its own 256 rows' feat slice
(for k), its adj rows [256,1024], W1/W2, and a pre-laid-out score weight A32s.

Per-core algorithm (i = destination row, j = source node, d = head dim 64):
  scores[i, j] = sum_d a[d] * relu(q[j, d] + k[i, d])
Layout trick: qT2 = [q^T; q^T] stacked [128(=2x64 d), 1024(=j)] in fp16.
For a PAIR of rows (2p, 2p+1), bias column kpair[:, p] = [k[2p]; k[2p+1]]:
  E2 = relu(qT2 + kpair[:, p])          one DVE tensor_scalar / ACT activation
  scores come from a PE matmul with lhsT = A32s slot q=p%16, an [128, 32]
  fp16 matrix holding `a` in column 2q (top d-half) and 2q+1 (bottom d-half),
  zeros elsewhere. 16 pairs accumulate into one 32-row psum band, so the
  matmul psum base stays 32-aligned (hardware requirement) while every
  logical row ends up at psum partition 2p+{0,1}.
Softmax row-wise with the mask folded in as att = exp(s - rowmax) * adj
(adj is 0/1, so this equals masking with -1e22 before softmax; rowmax of the
unmasked scores is a valid stabilizer since softmax is shift-invariant).
Final: out[i, :] = (att @ feat) / rowsum, att transposed on PE.
"""
import sys

sys.path.insert(0, "/opt/trn_rl_repo")

from contextlib import ExitStack

import numpy as np

import concourse.bass as bass  # noqa: F401
import concourse.tile as tile
from concourse import bacc, masks, mybir
from concourse.bass_utils import run_bass_kernel_spmd

B, T, C_IN, D = 2, 1024, 128, 64
N_CORES = 8
ROWS = (B * T) // N_CORES  # 256 destination rows per core
CPB = N_CORES // B  # cores per batch
NT = T // 128  # token tiles
NIT = ROWS // 128  # i-tiles per core
NPAIR = 64  # row pairs per i-tile
NSLOT = 16  # pair slots per 32-row psum band

FP32 = mybir.dt.float32
FP16 = mybir.dt.float16
AX = mybir.AxisListType.X
OP = mybir.AluOpType
AF = mybir.ActivationFunctionType

ACT_MOD = 4  # every ACT_MOD-th E2 tile is produced on ScalarE instead of VectorE


def _emit(ctx, tc, nc, feat, featk, adj, W1, W2, a32, out):
    singles = ctx.enter_context(tc.tile_pool(name="singles", bufs=1))
    ident = singles.tile([128, 128], FP32)
    masks.make_identity(nc, ident[:])
    feat_sb = singles.tile([128, NT * C_IN], FP32)  # block t = feat[t*128:(t+1)*128, :]
    qT2 = singles.tile([128, T], FP16)
    kpair = singles.tile([128, ROWS // 2], FP32)
    A32s = singles.tile([128, NSLOT * 32], FP16)
    nc.sync.dma_start(A32s[:], a32[:, :])

    with ExitStack() as sctx:
        spool = sctx.enter_context(tc.tile_pool(name="setup_sb", bufs=2))
        spsum = sctx.enter_context(tc.tile_pool(name="setup_ps", bufs=2, space="PSUM"))

        for t in range(NT):
            nc.sync.dma_start(feat_sb[:, t * 128 : (t + 1) * 128], feat[t * 128 : (t + 1) * 128, :])
        featT = spool.tile([128, T], FP32, tag="featT")
        for t in range(NT):
            ps = spsum.tile([128, 128], FP32, tag="tr")
            nc.tensor.transpose(ps[:], feat_sb[:, t * 128 : (t + 1) * 128], ident[:])
            nc.any.tensor_copy(featT[:, t * 128 : (t + 1) * 128], ps[:])

        featk_sb = spool.tile([128, (ROWS // 128) * C_IN], FP32, tag="featk")
        for t in range(ROWS // 128):
            nc.sync.dma_start(
                featk_sb[:, t * 128 : (t + 1) * 128], featk[t * 128 : (t + 1) * 128, :]
            )
        featkT = spool.tile([128, ROWS], FP32, tag="featkT")
        for t in range(ROWS // 128):
            ps = spsum.tile([128, 128], FP32, tag="tr")
            nc.tensor.transpose(ps[:], featk_sb[:, t * 128 : (t + 1) * 128], ident[:])
            nc.any.tensor_copy(featkT[:, t * 128 : (t + 1) * 128], ps[:])

        w_sb = spool.tile([64, 2 * C_IN], FP32, tag="w")
        nc.sync.dma_start(w_sb[:, 0:C_IN], W1[:, :])
        nc.sync.dma_start(w_sb[:, C_IN : 2 * C_IN], W2[:, :])
        wT = spool.tile([128, 2 * D], FP32, tag="wT")
        for wi in range(2):
            ps = spsum.tile([128, 128], FP32, tag="tr")
            nc.tensor.transpose(
                ps[:, 0:D], w_sb[:, wi * C_IN : (wi + 1) * C_IN], ident[0:64, 0:64]
            )
            nc.any.tensor_copy(wT[:, wi * D : (wi + 1) * D], ps[:, 0:D])

        # qT = W1 @ featT   [64, T] -> stacked fp16 qT2
        for h in range(T // 512):
            ps = spsum.tile([64, 512], FP32, tag="qk")
            nc.tensor.matmul(
                ps[:], wT[:, 0:D], featT[:, h * 512 : (h + 1) * 512], start=True, stop=True
            )
            nc.any.tensor_copy(qT2[0:64, h * 512 : (h + 1) * 512], ps[:])
        nc.vector.tensor_copy(qT2[64:128, :], qT2[0:64, :])

        # kT = W2 @ featkT  [64, ROWS] -> kpair
        kps = spsum.tile([64, ROWS], FP32, tag="qk")
        nc.tensor.matmul(kps[:], wT[:, D : 2 * D], featkT[:], start=True, stop=True)
        kT = spool.tile([64, ROWS], FP32, tag="kT")
        nc.any.tensor_copy(kT[:], kps[:])
        kTv = kT[:].rearrange("d (p two) -> d two p", two=2)
        nc.vector.tensor_copy(kpair[0:64, :], kTv[:, 0, :])
        nc.vector.tensor_copy(kpair[64:128, :], kTv[:, 1, :])

    e2pool = ctx.enter_context(tc.tile_pool(name="e2", bufs=4))
    adjpool = ctx.enter_context(tc.tile_pool(name="adjp", bufs=2))
    softpool = ctx.enter_context(tc.tile_pool(name="soft", bufs=2))
    smallpool = ctx.enter_context(tc.tile_pool(name="small", bufs=2))
    attTpool = ctx.enter_context(tc.tile_pool(name="attT", bufs=2))
    outpool = ctx.enter_context(tc.tile_pool(name="outp", bufs=2))
    ps_scores = ctx.enter_context(tc.tile_pool(name="ps_s", bufs=4, space="PSUM"))
    ps_tr = ctx.enter_context(tc.tile_pool(name="ps_tr", bufs=2, space="PSUM"))
    ps_out = ctx.enter_context(tc.tile_pool(name="ps_o", bufs=1, space="PSUM"))

    for it in range(NIT):
        adj_sb = adjpool.tile([128, T], FP32, tag="adj")
        nc.sync.dma_start(adj_sb[:], adj[it * 128 : (it + 1) * 128, :])

        s0 = ps_scores.tile([128, 512], FP32, tag="s")
        s1 = ps_scores.tile([128, 512], FP32, tag="s")
        for p in range(NPAIR):
            P = it * NPAIR + p
            e2 = e2pool.tile([128, T], FP16, tag="e2")
            kcol = kpair[:, P : P + 1]
            if p % ACT_MOD == ACT_MOD - 1:
                nc.scalar.activation(e2[:], qT2[:], AF.Relu, bias=kcol)
            else:
                nc.vector.tensor_scalar(e2[:], qT2[:], kcol, 0.0, OP.add, OP.max)
            g, q = divmod(p, NSLOT)
            lhsT = A32s[:, 32 * q : 32 * q + 32]
            first, last = q == 0, q == NSLOT - 1
            nc.tensor.matmul(
                s0[32 * g : 32 * g + 32, :],
                lhsT,
                e2[:, 0:512],
                start=first,
                stop=last,
                tile_position=(0, 32 * g),
            )
            nc.tensor.matmul(
                s1[32 * g : 32 * g + 32, :],
                lhsT,
                e2[:, 512:T],
                start=first,
                stop=last,
                tile_position=(0, 32 * g),
            )

        # row softmax with the mask folded in as a multiply by adj (0/1)
        m01 = smallpool.tile([128, 2], FP32, tag="m01")
        nc.vector.tensor_reduce(m01[:, 0:1], s0[:], AX, OP.max)
        nc.vector.tensor_reduce(m01[:, 1:2], s1[:], AX, OP.max)
        rmax = smallpool.tile([128, 1], FP32, tag="rmax")
        nc.vector.tensor_reduce(rmax[:], m01[:], AX, OP.max)
        negmax = smallpool.tile([128, 1], FP32, tag="negmax")
        nc.scalar.mul(negmax[:], rmax[:], -1.0)
        pexp = softpool.tile([128, T], FP32, tag="pexp")
        nc.scalar.activation(pexp[:, 0:512], s0[:], AF.Exp, bias=negmax[:])
        nc.scalar.activation(pexp[:, 512:T], s1[:], AF.Exp, bias=negmax[:])
        patt = softpool.tile([128, T], FP32, tag="patt")
        nc.vector.tensor_tensor(patt[:], pexp[:], adj_sb[:], OP.mult)
        rsum = smallpool.tile([128, 1], FP32, tag="rsum")
        nc.vector.tensor_reduce(rsum[:], patt[:], AX, OP.add)
        inv = smallpool.tile([128, 1], FP32, tag="inv")
        nc.vector.reciprocal(inv[:], rsum[:])

        attT = attTpool.tile([128, T], FP32, tag="attT")
        for t in range(NT):
            pst = ps_tr.tile([128, 128], FP32, tag="tr")
            nc.tensor.transpose(pst[:], patt[:, t * 128 : (t + 1) * 128], ident[:])
            nc.any.tensor_copy(attT[:, t * 128 : (t + 1) * 128], pst[:])

        po = ps_out.tile([128, C_IN], FP32, tag="o")
        for t in range(NT):
            nc.tensor.matmul(
                po[:],
                attT[:, t * 128 : (t + 1) * 128],
                feat_sb[:, t * 128 : (t + 1) * 128],
                start=(t == 0),
                stop=(t == NT - 1),
            )
        out_sb = outpool.tile([128, C_IN], FP32, tag="out")
        nc.vector.tensor_scalar(out_sb[:], po[:], inv[:], None, OP.mult)
        nc.sync.dma_start(out[it * 128 : (it + 1) * 128, :], out_sb[:])


_PROGRAM = None


def build_program():
    global _PROGRAM
    if _PROGRAM is not None:
        return _PROGRAM
    nc = bacc.Bacc("TRN2", target_bir_lowering=False, debug=False, num_devices=N_CORES)
    feat = nc.dram_tensor("feat", [T, C_IN], FP32, kind="ExternalInput")
    featk = nc.dram_tensor("featk", [ROWS, C_IN], FP32, kind="ExternalInput")
    adj = nc.dram_tensor("adj", [ROWS, T], FP32, kind="ExternalInput")
    W1 = nc.dram_tensor("W1", [D, C_IN], FP32, kind="ExternalInput")
    W2 = nc.dram_tensor("W2", [D, C_IN], FP32, kind="ExternalInput")
    a32 = nc.dram_tensor("a32", [128, NSLOT * 32], FP16, kind="ExternalInput")
    out = nc.dram_tensor("out", [ROWS, C_IN], FP32, kind="ExternalOutput")
    with tile.TileContext(nc) as tc:
        with ExitStack() as ctx:
            _emit(ctx, tc, nc, feat, featk, adj, W1, W2, a32, out)
    nc.compile()
    _PROGRAM = nc
    return nc


def make_a32(a):
    a32 = np.zeros((128, NSLOT * 32), dtype=np.float16)
    for q in range(NSLOT):
        a32[0:64, 32 * q + 2 * q] = a
        a32[64:128, 32 * q + 2 * q + 1] = a
    return a32


def make_in_maps(feat, adj, W1, W2, a):
    feat = np.ascontiguousarray(feat, dtype=np.float32)
    adj = np.ascontiguousarray(adj, dtype=np.float32)
    W1 = np.ascontiguousarray(W1, dtype=np.float32)
    W2 = np.ascontiguousarray(W2, dtype=np.float32)
    a32 = make_a32(np.asarray(a, dtype=np.float32))
    in_maps = []
    for c in range(N_CORES):
        b = c // CPB
        r0 = (c % CPB) * ROWS
        in_maps.append(
            {
                "feat": feat[b],
                "featk": np.ascontiguousarray(feat[b, r0 : r0 + ROWS]),
                "adj": np.ascontiguousarray(adj[b, r0 : r0 + ROWS]),
                "W1": W1,
                "W2": W2,
                "a32": a32,
            }
        )
    return in_maps


def run(feat, adj, W1, W2, a, trace=False):
    nc = build_program()
    in_maps = make_in_maps(feat, adj, W1, W2, a)
    res = run_bass_kernel_spmd(nc, in_maps, core_ids=list(range(N_CORES)), trace=trace)
    outs = [res.results[c]["out"] for c in range(N_CORES)]
    full = np.concatenate(outs, axis=0).reshape(B, T, C_IN).astype(np.float32)
    return full, res


def kernel(feat, adj, W1, W2, a):
    full, _ = run(feat, adj, W1, W2, a)
    return full
